# revision 32
# baseline (speedup 1.0000x reference)
"""MoE (top-2 of 16 routed experts + 2 shared experts) Trainium2 kernel.

Strategy: expert-parallel over 8 NeuronCores, token dispatch on host (the
router is 0.01% of the FLOPs; host-side routing lets each core receive
exactly the tokens it needs, already gathered, transposed and packed).

Per core (SPMD program, identical on all cores; per-core in_maps differ):
  slot "r1": routed expert (one of the 8 least-loaded), fp8 DoubleRow
  slot "r0": routed expert (one of the 8 most-loaded), fp8 DoubleRow
  slot "sh": one shared expert x one D-half x one token-half (2048 tokens),
             bf16
Each slot is a dense FFN in feature-major ("transposed") layout:
  mid^T[d,t] = gelu( sum_h Wup[h,d] * x^T[h,t] )
  y^T[h,t]   =       sum_d Wdn[d,h] * mid^T[d,t]
Host scatter-adds y^T into the [T,H] output with the router gate weights
(gelu is the only nonlinearity, so the per-token gate commutes with the
down projection).

Numerics: the routed contribution is ~19% of output energy, so fp8-e4m3
there (per-expert rel err ~5%) adds only ~1% to the total error; shared
experts stay bf16.  Routed weights are pre-scaled x32 into e4m3's normal
range; the up-projection's x32 is undone by the gelu activation scale,
the down-projection's x32 by dividing the host-side gate.

fp8 matmuls run in DoubleRow perf mode: lhsT [128,2,128] + rhs [128,2,w]
process a 256-deep contraction per instruction at 2 MAC/cell/cycle.  The
tile-major SBUF packing already pairs correctly: [:, 2b:2b+2, :] puts
contraction rows 256b+p and 256b+128+p on partition p.  Chunks of one
routed slot are interleaved in the b-loop so consecutive matmuls share a
stationary operand and LDWEIGHTS (~183ns for 256 cols) stays hidden.

All DRAM tensors are packed host-side as [128, free] with each partition's
data contiguous (x/y additionally chunk-major), so every DMA moves large
contiguous blocks per partition.
"""

import numpy as np
import ml_dtypes

import concourse.mybir as mybir
import concourse.tile as tile
from concourse import bacc
from concourse.bass_utils import run_bass_kernel_spmd

BF16 = mybir.dt.bfloat16
FP8 = mybir.dt.float8e4
F32 = mybir.dt.float32
NP_BF16 = ml_dtypes.bfloat16
NP_FP8 = ml_dtypes.float8_e4m3  # IEEE e4m3 (max 240) = TRN FP8_EXP4
GELU = mybir.ActivationFunctionType.Gelu
DR = mybir.MatmulPerfMode.DoubleRow

B, S, H, D = 4, 1024, 2048, 1024
T = B * S
E_RT, E_SH, CORES = 16, 2, 8
HT = H // 128        # contraction tiles for every up-projection (16)
DT_RT = D // 128     # routed intermediate tiles (8)
D_SH = D // 2        # per-core shared intermediate columns (512)
DT_SH = D_SH // 128  # 4
N_SH = T // 2        # shared-slot tokens per core (2048)
TT = 512             # max moving-operand / psum tile width
YG = 4               # output h-tiles staged per store DMA
WS = 32.0            # fp8 weight pre-scale (power of two)

_prog_cache = {}
LAST_RESULTS = None  # BassKernelResults of the most recent run (for test.py)


def _chunks(n):
    """Split n greedily into TT-wide chunks plus a runt.  Full-width
    chunks maximize matmul efficiency, and the single psum->y cast per
    512-wide chunk is cheaper on DVE than two medium casts."""
    return [(off, min(TT, n - off)) for off in range(0, n, TT)]


def _build_program(r0_cap, r1_cap):
    nc = bacc.Bacc("TRN2", target_bir_lowering=False, debug=False,
                   num_devices=CORES)
    # (name, n_tokens, fp8, dts);  contraction tiles are HT for all slots.
    # Order: r1 first (smallest load -> earliest compute start), sh in the
    # middle (its 12.6MB of loads hide under r1's compute), r0 last (its
    # final 288-wide chunks drain the tail fastest).
    slot_defs = [("r1", r1_cap, True, DT_RT),
                 ("sh", N_SH, False, DT_SH),
                 ("r0", r0_cap, True, DT_RT)]
    slots = []
    for name, n, fp8, dts in slot_defs:
        dt = FP8 if fp8 else BF16
        d_s = dts * 128
        xd = nc.dram_tensor(f"x_{name}", [128, HT * n], dt,
                            kind="ExternalInput")
        wu = nc.dram_tensor(f"wup_{name}", [128, HT * d_s], dt,
                            kind="ExternalInput")
        wd = nc.dram_tensor(f"wdn_{name}", [128, dts * H], dt,
                            kind="ExternalInput")
        yd = nc.dram_tensor(f"y_{name}", [128, HT * n], BF16,
                            kind="ExternalOutput")
        slots.append((name, n, fp8, dts, xd, wu, wd, yd))

    with tile.TileContext(nc) as tc:
        with (
            tc.tile_pool(name="wpool", bufs=1) as wpool,
            tc.tile_pool(name="xpool", bufs=1) as xpool,
            tc.tile_pool(name="mpool", bufs=1) as mpool,
            tc.tile_pool(name="ypool", bufs=4) as ypool,
            tc.tile_pool(name="pspool", bufs=8, space="PSUM") as pspool,
        ):
            # DMA orchestration.  Two HWDGE rings exist (issued via SP and
            # ACT); each processes its DMAs in issue order, and an issue
            # blocks while its destination pool slot is busy.  The rings
            # run at ~180 GB/s each (HBM aggregate ~360), so the startup
            # critical path is balanced across both: r1's x+wup split
            # evenly, then sh's critical pieces, then r0's loads with
            # sh's trailing x chunks interleaved behind them.  The ACT
            # ring additionally carries all y stores.  Weight/x buffers
            # are statically dedicated per slot (per-slot tags) except
            # sh's x, which round-robins 3 buffers.
            def slot_io(si):
                name, n, fp8, dts, xd, wu, wd, yd = slots[si]
                dt = FP8 if fp8 else BF16
                d_s = dts * 128
                chs = _chunks(n)
                wut = wpool.tile([128, HT, d_s], dt, tag=f"wup_{name}",
                                 name=f"wup_{name}")
                wdt = wpool.tile([128, dts, H], dt, tag=f"wdn_{name}",
                                 name=f"wdn_{name}")
                xts = [None] * len(chs)

                def load_x(ci, eng, h0=0, hn=HT):
                    off, w = chs[ci]
                    if xts[ci] is None:
                        bufs = 3 if name == "sh" else 1
                        xtag = (f"x_{name}" if name == "sh"
                                else f"x_{name}_{ci}")
                        xts[ci] = xpool.tile([128, HT, w], dt, tag=xtag,
                                             bufs=bufs,
                                             name=f"x_{name}_{off}")
                    eng.dma_start(
                        out=xts[ci][:, h0:h0 + hn, :],
                        in_=xd[:, HT * off + h0 * w:
                               HT * off + (h0 + hn) * w].rearrange(
                            "p (h w) -> p h w", h=hn))

                def load_wup(gi, eng, hg=4):
                    g = gi * hg
                    eng.dma_start(
                        out=wut[:, g:g + hg, :],
                        in_=wu[:, g * d_s:(g + hg) * d_s].rearrange(
                            "p (h d) -> p h d", h=hg))

                def load_wup_dh(dh, gi, eng, hg=4):
                    # fp8 wup DRAM layout is D-half-major ([2, HT, 512] per
                    # partition) so the first up-chains only need half the
                    # weight bytes before they can run to completion.
                    g = gi * hg
                    base = (dh * HT + g) * 512
                    eng.dma_start(
                        out=wut[:, g:g + hg, dh * 512:(dh + 1) * 512],
                        in_=wu[:, base:base + hg * 512].rearrange(
                            "p (h d) -> p h d", h=hg))

                def load_wdn(gi, eng):
                    dg = 4
                    g = gi * dg
                    eng.dma_start(
                        out=wdt[:, g:g + dg, :],
                        in_=wd[:, g * H:(g + dg) * H].rearrange(
                            "p (c h) -> p c h", c=dg))

                return (wut, wdt, xts, load_x, load_wup, load_wdn,
                        load_wup_dh)

            # Dummy matmuls on scratch tiles: the PE HAM clock-gate only
            # lifts to 2.4 GHz after ~3.4us of sustained activity, so warm
            # it up while the first loads stream in.
            wlhs = xpool.tile([128, 128], BF16, tag="warm_l", bufs=1,
                              name="warm_lhs")
            wrhs = xpool.tile([128, TT], BF16, tag="warm_r", bufs=1,
                              name="warm_rhs")
            nc.vector.memset(wlhs[:], 0)
            nc.vector.memset(wrhs[:], 0)
            wps = pspool.tile([128, TT], F32, tag="ps", name="warm_ps")
            for wi in range(23):
                nc.tensor.matmul(wps[:], lhsT=wlhs[:], rhs=wrhs[:],
                                 start=True, stop=True)

            sio = [slot_io(si) for si in range(len(slots))]
            (_, _, _, r1_x, _, r1_wdn, r1_wup) = sio[0]
            (_, _, _, sh_x, sh_wup, sh_wdn, _) = sio[1]
            (_, _, _, r0_x, _, r0_wdn, r0_wup) = sio[2]
            hh = HT // 2
            # r1 (first slot): balanced across both rings in consumption
            # order -- x + the first D-half of wup lets the first 4
            # up-chains run to completion, then the second half, then wdn
            r1_x(0, nc.sync, 0, hh)
            r1_wup(0, 0, nc.scalar)
            r1_wup(0, 1, nc.sync)
            r1_x(0, nc.scalar, hh, HT - hh)
            r1_wup(0, 2, nc.scalar)
            r1_wup(0, 3, nc.sync)
            r1_wup(1, 0, nc.scalar)
            r1_wup(1, 1, nc.sync)
            r1_wup(1, 2, nc.scalar)
            r1_wup(1, 3, nc.sync)
            r1_wdn(0, nc.sync)
            r1_wdn(1, nc.scalar)
            # sh: critical pieces (wup + x chunk 0) split across rings
            sh_wup(0, nc.sync)
            sh_wup(1, nc.scalar)
            sh_wup(2, nc.sync)
            sh_wup(3, nc.scalar)
            sh_x(0, nc.sync, hh, HT - hh)
            sh_x(0, nc.scalar, 0, hh)
            sh_wdn(0, nc.sync)
            sh_x(1, nc.sync)
            # r0's loads, with sh's trailing x chunks interleaved behind
            for dh in range(2):
                for gi in range(4):
                    r0_wup(dh, gi, nc.sync)
            r0_x(0, nc.sync)
            sh_x(2, nc.sync)
            r0_wdn(0, nc.sync)
            r0_wdn(1, nc.sync)
            r0_x(1, nc.sync)
            sh_x(3, nc.sync)

            def store_y(yd, yt, off, hi, w):
                lo = HT * off + (hi - (YG - 1)) * w
                nc.scalar.dma_start(
                    out=yd[:, lo:lo + YG * w].rearrange(
                        "p (h w) -> p h w", h=YG),
                    in_=yt[:])

            def routed_compute(name, n, dts, xts, wut, wdt, yd,
                               last=False):
                chs = _chunks(n)
                nch = len(chs)
                mids = [mpool.tile([128, dts, w], FP8, tag=f"mid_{name}",
                                   bufs=nch, name=f"mid_{name}_{off}")
                        for off, w in chs]
                hb = HT // 2
                for dj in range(dts):
                    pss = [pspool.tile([128, TT], F32, tag="ps",
                                        name=f"p1_{name}_{dj}_{ci}")
                           for ci in range(nch)]
                    for b in range(hb):
                        for ci, (off, w) in enumerate(chs):
                            nc.tensor.matmul(
                                pss[ci][:, :w],
                                lhsT=wut[:, 2 * b:2 * b + 2,
                                         dj * 128:(dj + 1) * 128],
                                rhs=xts[ci][:, 2 * b:2 * b + 2, :],
                                start=(b == 0), stop=(b == hb - 1),
                                perf_mode=DR)
                    for ci, (off, w) in enumerate(chs):
                        nc.scalar.activation(mids[ci][:, dj, :],
                                             pss[ci][:, :w], GELU,
                                             scale=1.0 / WS)
                db = dts // 2
                yts = [None] * nch
                for hi in range(HT):
                    ps2s = [pspool.tile([128, TT], F32, tag="ps",
                                         name=f"p2_{name}_{hi}_{ci}")
                            for ci in range(nch)]
                    for c in range(db):
                        for ci, (off, w) in enumerate(chs):
                            nc.tensor.matmul(
                                ps2s[ci][:, :w],
                                lhsT=wdt[:, 2 * c:2 * c + 2,
                                         hi * 128:(hi + 1) * 128],
                                rhs=mids[ci][:, 2 * c:2 * c + 2, :],
                                start=(c == 0), stop=(c == db - 1),
                                perf_mode=DR)
                    g = hi % YG
                    for ci, (off, w) in enumerate(chs):
                        if g == 0:
                            yts[ci] = ypool.tile([128, YG, w], BF16,
                                                 tag="y",
                                                 name=f"y_{name}_{hi}_{ci}")
                        nc.vector.tensor_copy(yts[ci][:, g, :],
                                              ps2s[ci][:, :w])
                        if last and hi >= HT - YG:
                            # final group of the final slot: store per-hi
                            # so the kernel's last store DMA is small and
                            # the end-of-kernel DMA-completion barrier
                            # fires sooner
                            lo = HT * off + hi * w
                            nc.scalar.dma_start(
                                out=yd[:, lo:lo + w].rearrange(
                                    "p (h w) -> p h w", h=1),
                                in_=yts[ci][:, g:g + 1, :])
                        elif g == YG - 1:
                            store_y(yd, yts[ci], off, hi, w)

            def shared_compute(name, n, dts, xts, wut, wdt, yd):
                for ci, (off, w) in enumerate(_chunks(n)):
                    xt = xts[ci]
                    mid = mpool.tile([128, dts, w], BF16, tag="mid_sh",
                                     bufs=2, name=f"mid_{name}_{off}")
                    for dj in range(dts):
                        ps = pspool.tile([128, TT], F32, tag="ps",
                                          name=f"p1_{name}_{off}_{dj}")
                        for hi in range(HT):
                            nc.tensor.matmul(
                                ps[:, :w],
                                lhsT=wut[:, hi, dj * 128:(dj + 1) * 128],
                                rhs=xt[:, hi, :],
                                start=(hi == 0), stop=(hi == HT - 1))
                        nc.scalar.activation(mid[:, dj, :], ps[:, :w],
                                             GELU)
                    yt = None
                    for hi in range(HT):
                        ps2 = pspool.tile([128, TT], F32, tag="ps",
                                           name=f"p2_{name}_{off}_{hi}")
                        for dj in range(dts):
                            nc.tensor.matmul(
                                ps2[:, :w],
                                lhsT=wdt[:, dj, hi * 128:(hi + 1) * 128],
                                rhs=mid[:, dj, :],
                                start=(dj == 0), stop=(dj == dts - 1))
                        g = hi % YG
                        if g == 0:
                            yt = ypool.tile([128, YG, w], BF16, tag="y",
                                            name=f"y_{name}_{off}_{hi}")
                        if hi % 2:
                            nc.scalar.copy(yt[:, g, :], ps2[:, :w])
                        else:
                            nc.vector.tensor_copy(yt[:, g, :], ps2[:, :w])
                        if g == YG - 1:
                            store_y(yd, yt, off, hi, w)

            for si, (name, n, fp8, dts, xd, wu, wd, yd) in enumerate(slots):
                wut, wdt, xts = sio[si][:3]
                if fp8:
                    routed_compute(name, n, dts, xts, wut, wdt, yd,
                                   last=(si == len(slots) - 1))
                else:
                    shared_compute(name, n, dts, xts, wut, wdt, yd)
    nc.compile()
    return nc


def _pack_rows(a, nt):
    """[nt*128, m] row-major -> [128, nt*m] with per-partition contiguous
    (tile-major) layout."""
    m = a.shape[1]
    return np.ascontiguousarray(
        a.reshape(nt, 128, m).transpose(1, 0, 2).reshape(128, nt * m))


def _pack_x(xTc):
    """[H, n] -> [128, HT*n] chunk-major."""
    n = xTc.shape[1]
    parts = [_pack_rows(xTc[:, off:off + w], HT) for off, w in _chunks(n)]
    return np.ascontiguousarray(np.concatenate(parts, axis=1))


def _unpack_y(yflat, n):
    """[128, HT*n] chunk-major -> [n, H] (token-major)."""
    yflat = yflat.astype(np.float32)
    out = np.empty((n, H), np.float32)
    base = 0
    for off, w in _chunks(n):
        blk = yflat[:, base:base + HT * w].reshape(128, HT, w)
        out[off:off + w] = blk.transpose(2, 1, 0).reshape(w, H)
        base += HT * w
    return out


def _route(x2d, w_router):
    """Top-2 routing, matching the reference's softmax-then-top_k."""
    logits = x2d @ w_router
    m = logits.max(-1, keepdims=True)
    e = np.exp(logits - m)
    probs = e / e.sum(-1, keepdims=True)
    rows = np.arange(x2d.shape[0])
    i1 = probs.argmax(-1)
    masked = probs.copy()
    masked[rows, i1] = -np.inf
    i2 = masked.argmax(-1)
    return probs, i1, i2


def kernel(x, Wsh_up, Wsh_down, Wrt_up, Wrt_down, W_router):
    global LAST_RESULTS
    x = np.asarray(x, np.float32)
    Wsh_up = np.asarray(Wsh_up, np.float32)
    Wsh_down = np.asarray(Wsh_down, np.float32)
    Wrt_up = np.asarray(Wrt_up, np.float32)
    Wrt_down = np.asarray(Wrt_down, np.float32)
    W_router = np.asarray(W_router, np.float32)

    x2d = x.reshape(T, H)
    probs, i1, i2 = _route(x2d, W_router)

    # token ids / gate values per routed expert
    ids, gates = [], []
    for e in range(E_RT):
        sel = np.where((i1 == e) | (i2 == e))[0]
        ids.append(sel)
        gates.append(probs[sel, e].astype(np.float32))

    # slot r0 takes the 8 most-loaded experts, r1 the 8 least-loaded, so
    # the two static capacities hug the actual counts.
    order = sorted(range(E_RT), key=lambda e: -len(ids[e]))
    slot_experts = {0: order[:CORES], 1: order[CORES:]}
    caps = []
    for slot in range(2):
        mx = max(len(ids[e]) for e in slot_experts[slot])
        caps.append(max(512, -(-mx // 16) * 16))
    r0_cap, r1_cap = caps

    key = (r0_cap, r1_cap)
    if key not in _prog_cache:
        _prog_cache[key] = _build_program(r0_cap, r1_cap)
    nc = _prog_cache[key]

    xbf = x2d.astype(NP_BF16)
    x8 = x2d.astype(NP_FP8)  # |x| < 6 << 240: no clipping needed

    in_maps = []
    for c in range(CORES):
        se, dh, th = c % E_SH, (c // E_SH) % 2, c // 4
        m = {
            "x_sh": _pack_x(np.ascontiguousarray(
                xbf[th * N_SH:(th + 1) * N_SH].T)),
            "wup_sh": _pack_rows(np.ascontiguousarray(
                Wsh_up[se][:, dh * D_SH:(dh + 1) * D_SH]).astype(NP_BF16),
                HT),
            "wdn_sh": _pack_rows(np.ascontiguousarray(
                Wsh_down[se][dh * D_SH:(dh + 1) * D_SH, :]).astype(NP_BF16),
                DT_SH),
        }
        for slot, cap in ((0, r0_cap), (1, r1_cap)):
            e = slot_experts[slot][c]
            sel = ids[e]
            xe = np.zeros((H, cap), NP_FP8)
            xe[:, :len(sel)] = x8[sel].T
            m[f"x_r{slot}"] = _pack_x(xe)
            wu8 = (Wrt_up[e] * WS).astype(NP_FP8)
            # D-half-major layout: [2, HT, 512] per partition (see
            # load_wup_dh)
            m[f"wup_r{slot}"] = _pack_rows(
                np.ascontiguousarray(np.vstack([wu8[:, :512],
                                                wu8[:, 512:]])), 2 * HT)
            m[f"wdn_r{slot}"] = _pack_rows(
                (Wrt_down[e] * WS).astype(NP_FP8), DT_RT)
        in_maps.append(m)

    res = run_bass_kernel_spmd(nc, in_maps, core_ids=list(range(CORES)))
    LAST_RESULTS = res

    out = np.zeros((T, H), np.float32)
    for c in range(CORES):
        th = c // 4
        out[th * N_SH:(th + 1) * N_SH] += _unpack_y(res.results[c]["y_sh"],
                                                    N_SH)
    for slot, cap in ((0, r0_cap), (1, r1_cap)):
        for c in range(CORES):
            e = slot_experts[slot][c]
            sel = ids[e]
            y = _unpack_y(res.results[c][f"y_r{slot}"], cap)
            out[sel] += (gates[e][:, None] / WS) * y[:len(sel)]
    return out.reshape(B, S, H)


# revision 34
# speedup vs baseline: 1.0204x; 1.0204x over previous
"""MoE (top-2 of 16 routed experts + 2 shared experts) Trainium2 kernel.

Strategy: expert-parallel over 8 NeuronCores, token dispatch on host (the
router is 0.01% of the FLOPs; host-side routing lets each core receive
exactly the tokens it needs, already gathered, transposed and packed).

Per core (SPMD program, identical on all cores; per-core in_maps differ):
  slot "r1": routed expert (one of the 8 least-loaded), fp8 DoubleRow
  slot "r0": routed expert (one of the 8 most-loaded), fp8 DoubleRow
  slot "sh": one shared expert x one D-half x one token-half (2048 tokens),
             bf16
Each slot is a dense FFN in feature-major ("transposed") layout:
  mid^T[d,t] = gelu( sum_h Wup[h,d] * x^T[h,t] )
  y^T[h,t]   =       sum_d Wdn[d,h] * mid^T[d,t]
Host scatter-adds y^T into the [T,H] output with the router gate weights
(gelu is the only nonlinearity, so the per-token gate commutes with the
down projection).

Numerics: the routed contribution is ~19% of output energy, so fp8-e4m3
there (per-expert rel err ~5%) adds only ~1% to the total error; shared
experts stay bf16.  Routed weights are pre-scaled x32 into e4m3's normal
range; the up-projection's x32 is undone by the gelu activation scale,
the down-projection's x32 by dividing the host-side gate.

fp8 matmuls run in DoubleRow perf mode: lhsT [128,2,128] + rhs [128,2,w]
process a 256-deep contraction per instruction at 2 MAC/cell/cycle.  The
tile-major SBUF packing already pairs correctly: [:, 2b:2b+2, :] puts
contraction rows 256b+p and 256b+128+p on partition p.  Chunks of one
routed slot are interleaved in the b-loop so consecutive matmuls share a
stationary operand and LDWEIGHTS (~183ns for 256 cols) stays hidden.

All DRAM tensors are packed host-side as [128, free] with each partition's
data contiguous (x/y additionally chunk-major), so every DMA moves large
contiguous blocks per partition.
"""

import numpy as np
import ml_dtypes

import concourse.mybir as mybir
import concourse.tile as tile
from concourse import bacc
from concourse.bass_utils import run_bass_kernel_spmd

BF16 = mybir.dt.bfloat16
FP8 = mybir.dt.float8e4
F32 = mybir.dt.float32
NP_BF16 = ml_dtypes.bfloat16
NP_FP8 = ml_dtypes.float8_e4m3  # IEEE e4m3 (max 240) = TRN FP8_EXP4
GELU = mybir.ActivationFunctionType.Gelu
DR = mybir.MatmulPerfMode.DoubleRow

B, S, H, D = 4, 1024, 2048, 1024
T = B * S
E_RT, E_SH, CORES = 16, 2, 8
HT = H // 128        # contraction tiles for every up-projection (16)
DT_RT = D // 128     # routed intermediate tiles (8)
D_SH = D // 2        # per-core shared intermediate columns (512)
DT_SH = D_SH // 128  # 4
N_SH = T // 2        # shared-slot tokens per core (2048)
TT = 512             # max moving-operand / psum tile width
YG = 4               # output h-tiles staged per store DMA
WS = 32.0            # fp8 weight pre-scale (power of two)

_prog_cache = {}
LAST_RESULTS = None  # BassKernelResults of the most recent run (for test.py)


def _chunks(n):
    """Split n greedily into TT-wide chunks plus a runt.  Full-width
    chunks maximize matmul efficiency, and the single psum->y cast per
    512-wide chunk is cheaper on DVE than two medium casts."""
    return [(off, min(TT, n - off)) for off in range(0, n, TT)]


def _build_program(r0_cap, r1_cap):
    nc = bacc.Bacc("TRN2", target_bir_lowering=False, debug=False,
                   num_devices=CORES)
    # (name, n_tokens, fp8, dts);  contraction tiles are HT for all slots.
    # Order: r1 first (smallest load -> earliest compute start), sh in the
    # middle (its 12.6MB of loads hide under r1's compute), r0 last (its
    # final 288-wide chunks drain the tail fastest).
    slot_defs = [("r1", r1_cap, True, DT_RT),
                 ("sh", N_SH, False, DT_SH),
                 ("r0", r0_cap, True, DT_RT)]
    slots = []
    for name, n, fp8, dts in slot_defs:
        dt = FP8 if fp8 else BF16
        d_s = dts * 128
        xd = nc.dram_tensor(f"x_{name}", [128, HT * n], dt,
                            kind="ExternalInput")
        wu = nc.dram_tensor(f"wup_{name}", [128, HT * d_s], dt,
                            kind="ExternalInput")
        wd = nc.dram_tensor(f"wdn_{name}", [128, dts * H], dt,
                            kind="ExternalInput")
        yd = nc.dram_tensor(f"y_{name}", [128, HT * n], BF16,
                            kind="ExternalOutput")
        slots.append((name, n, fp8, dts, xd, wu, wd, yd))

    with tile.TileContext(nc) as tc:
        with (
            tc.tile_pool(name="wpool", bufs=1) as wpool,
            tc.tile_pool(name="xpool", bufs=1) as xpool,
            tc.tile_pool(name="mpool", bufs=1) as mpool,
            tc.tile_pool(name="ypool", bufs=4) as ypool,
            tc.tile_pool(name="pspool", bufs=8, space="PSUM") as pspool,
        ):
            # DMA orchestration.  Two HWDGE rings exist (issued via SP and
            # ACT); each processes its DMAs in issue order, and an issue
            # blocks while its destination pool slot is busy.  The rings
            # run at ~180 GB/s each (HBM aggregate ~360), so the startup
            # critical path is balanced across both: r1's x+wup split
            # evenly, then sh's critical pieces, then r0's loads with
            # sh's trailing x chunks interleaved behind them.  The ACT
            # ring additionally carries all y stores.  Weight/x buffers
            # are statically dedicated per slot (per-slot tags) except
            # sh's x, which round-robins 3 buffers.
            def slot_io(si):
                name, n, fp8, dts, xd, wu, wd, yd = slots[si]
                dt = FP8 if fp8 else BF16
                d_s = dts * 128
                chs = _chunks(n)
                wut = wpool.tile([128, HT, d_s], dt, tag=f"wup_{name}",
                                 name=f"wup_{name}")
                wdt = wpool.tile([128, dts, H], dt, tag=f"wdn_{name}",
                                 name=f"wdn_{name}")
                xts = [None] * len(chs)

                def load_x(ci, eng, h0=0, hn=HT):
                    off, w = chs[ci]
                    if xts[ci] is None:
                        bufs = 3 if name == "sh" else 1
                        xtag = (f"x_{name}" if name == "sh"
                                else f"x_{name}_{ci}")
                        xts[ci] = xpool.tile([128, HT, w], dt, tag=xtag,
                                             bufs=bufs,
                                             name=f"x_{name}_{off}")
                    eng.dma_start(
                        out=xts[ci][:, h0:h0 + hn, :],
                        in_=xd[:, HT * off + h0 * w:
                               HT * off + (h0 + hn) * w].rearrange(
                            "p (h w) -> p h w", h=hn))

                def load_wup(gi, eng, hg=4):
                    g = gi * hg
                    eng.dma_start(
                        out=wut[:, g:g + hg, :],
                        in_=wu[:, g * d_s:(g + hg) * d_s].rearrange(
                            "p (h d) -> p h d", h=hg))

                def load_wup_dh(dh, gi, eng, hg=4):
                    # fp8 wup DRAM layout is D-half-major ([2, HT, 512] per
                    # partition) so the first up-chains only need half the
                    # weight bytes before they can run to completion.
                    g = gi * hg
                    base = (dh * HT + g) * 512
                    eng.dma_start(
                        out=wut[:, g:g + hg, dh * 512:(dh + 1) * 512],
                        in_=wu[:, base:base + hg * 512].rearrange(
                            "p (h d) -> p h d", h=hg))

                def load_wdn(gi, eng):
                    dg = 4
                    g = gi * dg
                    eng.dma_start(
                        out=wdt[:, g:g + dg, :],
                        in_=wd[:, g * H:(g + dg) * H].rearrange(
                            "p (c h) -> p c h", c=dg))

                return (wut, wdt, xts, load_x, load_wup, load_wdn,
                        load_wup_dh)

            # Dummy matmuls on scratch tiles: the PE HAM clock-gate only
            # lifts to 2.4 GHz after ~3.4us of sustained activity, so warm
            # it up while the first loads stream in.
            wlhs = xpool.tile([128, 128], BF16, tag="warm_l", bufs=1,
                              name="warm_lhs")
            wrhs = xpool.tile([128, TT], BF16, tag="warm_r", bufs=1,
                              name="warm_rhs")
            nc.vector.memset(wlhs[:], 0)
            nc.vector.memset(wrhs[:], 0)
            wps = pspool.tile([128, TT], F32, tag="ps", name="warm_ps")
            for wi in range(20):
                nc.tensor.matmul(wps[:], lhsT=wlhs[:], rhs=wrhs[:],
                                 start=True, stop=True)

            sio = [slot_io(si) for si in range(len(slots))]
            (_, _, _, r1_x, _, r1_wdn, r1_wup) = sio[0]
            (_, _, _, sh_x, sh_wup, sh_wdn, _) = sio[1]
            (_, _, _, r0_x, _, r0_wdn, r0_wup) = sio[2]
            hh = HT // 2
            # r1 (first slot): balanced across both rings in consumption
            # order -- x + the first D-half of wup lets the first 4
            # up-chains run to completion, then the second half, then wdn
            r1_x(0, nc.sync, 0, hh)
            r1_wup(0, 0, nc.scalar)
            r1_wup(0, 1, nc.sync)
            r1_x(0, nc.scalar, hh, HT - hh)
            r1_wup(0, 2, nc.scalar)
            r1_wup(0, 3, nc.sync)
            r1_wup(1, 0, nc.scalar)
            r1_wup(1, 1, nc.sync)
            r1_wup(1, 2, nc.scalar)
            r1_wup(1, 3, nc.sync)
            r1_wdn(0, nc.sync)
            r1_wdn(1, nc.scalar)
            # sh: critical pieces (wup + x chunk 0) split across rings
            sh_wup(0, nc.sync)
            sh_wup(1, nc.scalar)
            sh_wup(2, nc.sync)
            sh_wup(3, nc.scalar)
            sh_x(0, nc.sync, hh, HT - hh)
            sh_x(0, nc.scalar, 0, hh)
            sh_wdn(0, nc.sync)
            sh_x(1, nc.sync)
            # r0's loads, with sh's trailing x chunks interleaved behind
            for dh in range(2):
                for gi in range(4):
                    r0_wup(dh, gi, nc.sync)
            r0_x(0, nc.sync)
            sh_x(2, nc.sync)
            r0_wdn(0, nc.sync)
            r0_wdn(1, nc.sync)
            r0_x(1, nc.sync)
            sh_x(3, nc.sync)

            def store_y(yd, yt, off, hi, w):
                lo = HT * off + (hi - (YG - 1)) * w
                nc.scalar.dma_start(
                    out=yd[:, lo:lo + YG * w].rearrange(
                        "p (h w) -> p h w", h=YG),
                    in_=yt[:])

            def routed_compute(name, n, dts, xts, wut, wdt, yd,
                               last=False):
                chs = _chunks(n)
                nch = len(chs)
                mids = [mpool.tile([128, dts, w], FP8, tag=f"mid_{name}",
                                   bufs=nch, name=f"mid_{name}_{off}")
                        for off, w in chs]
                hb = HT // 2
                for dj in range(dts):
                    pss = [pspool.tile([128, TT], F32, tag="ps",
                                        name=f"p1_{name}_{dj}_{ci}")
                           for ci in range(nch)]
                    for b in range(hb):
                        for ci, (off, w) in enumerate(chs):
                            nc.tensor.matmul(
                                pss[ci][:, :w],
                                lhsT=wut[:, 2 * b:2 * b + 2,
                                         dj * 128:(dj + 1) * 128],
                                rhs=xts[ci][:, 2 * b:2 * b + 2, :],
                                start=(b == 0), stop=(b == hb - 1),
                                perf_mode=DR)
                    for ci, (off, w) in enumerate(chs):
                        nc.scalar.activation(mids[ci][:, dj, :],
                                             pss[ci][:, :w], GELU,
                                             scale=1.0 / WS)
                db = dts // 2
                yts = [None] * nch
                for hi in range(HT):
                    ps2s = [pspool.tile([128, TT], F32, tag="ps",
                                         name=f"p2_{name}_{hi}_{ci}")
                            for ci in range(nch)]
                    for c in range(db):
                        for ci, (off, w) in enumerate(chs):
                            nc.tensor.matmul(
                                ps2s[ci][:, :w],
                                lhsT=wdt[:, 2 * c:2 * c + 2,
                                         hi * 128:(hi + 1) * 128],
                                rhs=mids[ci][:, 2 * c:2 * c + 2, :],
                                start=(c == 0), stop=(c == db - 1),
                                perf_mode=DR)
                    g = hi % YG
                    for ci, (off, w) in enumerate(chs):
                        if g == 0:
                            yts[ci] = ypool.tile([128, YG, w], BF16,
                                                 tag="y",
                                                 name=f"y_{name}_{hi}_{ci}")
                        nc.vector.tensor_copy(yts[ci][:, g, :],
                                              ps2s[ci][:, :w])
                        if last and hi >= HT - YG:
                            # final group of the final slot: store in
                            # half-groups so the kernel's last store DMA
                            # is smaller and the end-of-kernel
                            # DMA-completion barrier fires sooner
                            if g % 2:
                                lo = HT * off + (hi - 1) * w
                                nc.scalar.dma_start(
                                    out=yd[:, lo:lo + 2 * w].rearrange(
                                        "p (h w) -> p h w", h=2),
                                    in_=yts[ci][:, g - 1:g + 1, :])
                        elif g == YG - 1:
                            store_y(yd, yts[ci], off, hi, w)

            def shared_compute(name, n, dts, xts, wut, wdt, yd):
                for ci, (off, w) in enumerate(_chunks(n)):
                    xt = xts[ci]
                    mid = mpool.tile([128, dts, w], BF16, tag="mid_sh",
                                     bufs=2, name=f"mid_{name}_{off}")
                    for dj in range(dts):
                        ps = pspool.tile([128, TT], F32, tag="ps",
                                          name=f"p1_{name}_{off}_{dj}")
                        for hi in range(HT):
                            nc.tensor.matmul(
                                ps[:, :w],
                                lhsT=wut[:, hi, dj * 128:(dj + 1) * 128],
                                rhs=xt[:, hi, :],
                                start=(hi == 0), stop=(hi == HT - 1))
                        nc.scalar.activation(mid[:, dj, :], ps[:, :w],
                                             GELU)
                    yt = None
                    for hi in range(HT):
                        ps2 = pspool.tile([128, TT], F32, tag="ps",
                                           name=f"p2_{name}_{off}_{hi}")
                        for dj in range(dts):
                            nc.tensor.matmul(
                                ps2[:, :w],
                                lhsT=wdt[:, dj, hi * 128:(hi + 1) * 128],
                                rhs=mid[:, dj, :],
                                start=(dj == 0), stop=(dj == dts - 1))
                        g = hi % YG
                        if g == 0:
                            yt = ypool.tile([128, YG, w], BF16, tag="y",
                                            name=f"y_{name}_{off}_{hi}")
                        if hi % 2:
                            nc.scalar.copy(yt[:, g, :], ps2[:, :w])
                        else:
                            nc.vector.tensor_copy(yt[:, g, :], ps2[:, :w])
                        if g == YG - 1:
                            store_y(yd, yt, off, hi, w)

            for si, (name, n, fp8, dts, xd, wu, wd, yd) in enumerate(slots):
                wut, wdt, xts = sio[si][:3]
                if fp8:
                    routed_compute(name, n, dts, xts, wut, wdt, yd,
                                   last=(si == len(slots) - 1))
                else:
                    shared_compute(name, n, dts, xts, wut, wdt, yd)
    nc.compile()
    return nc


def _pack_rows(a, nt):
    """[nt*128, m] row-major -> [128, nt*m] with per-partition contiguous
    (tile-major) layout."""
    m = a.shape[1]
    return np.ascontiguousarray(
        a.reshape(nt, 128, m).transpose(1, 0, 2).reshape(128, nt * m))


def _pack_x(xTc):
    """[H, n] -> [128, HT*n] chunk-major."""
    n = xTc.shape[1]
    parts = [_pack_rows(xTc[:, off:off + w], HT) for off, w in _chunks(n)]
    return np.ascontiguousarray(np.concatenate(parts, axis=1))


def _unpack_y(yflat, n):
    """[128, HT*n] chunk-major -> [n, H] (token-major)."""
    yflat = yflat.astype(np.float32)
    out = np.empty((n, H), np.float32)
    base = 0
    for off, w in _chunks(n):
        blk = yflat[:, base:base + HT * w].reshape(128, HT, w)
        out[off:off + w] = blk.transpose(2, 1, 0).reshape(w, H)
        base += HT * w
    return out


def _route(x2d, w_router):
    """Top-2 routing, matching the reference's softmax-then-top_k."""
    logits = x2d @ w_router
    m = logits.max(-1, keepdims=True)
    e = np.exp(logits - m)
    probs = e / e.sum(-1, keepdims=True)
    rows = np.arange(x2d.shape[0])
    i1 = probs.argmax(-1)
    masked = probs.copy()
    masked[rows, i1] = -np.inf
    i2 = masked.argmax(-1)
    return probs, i1, i2


def kernel(x, Wsh_up, Wsh_down, Wrt_up, Wrt_down, W_router):
    global LAST_RESULTS
    x = np.asarray(x, np.float32)
    Wsh_up = np.asarray(Wsh_up, np.float32)
    Wsh_down = np.asarray(Wsh_down, np.float32)
    Wrt_up = np.asarray(Wrt_up, np.float32)
    Wrt_down = np.asarray(Wrt_down, np.float32)
    W_router = np.asarray(W_router, np.float32)

    x2d = x.reshape(T, H)
    probs, i1, i2 = _route(x2d, W_router)

    # token ids / gate values per routed expert
    ids, gates = [], []
    for e in range(E_RT):
        sel = np.where((i1 == e) | (i2 == e))[0]
        ids.append(sel)
        gates.append(probs[sel, e].astype(np.float32))

    # slot r0 takes the 8 most-loaded experts, r1 the 8 least-loaded, so
    # the two static capacities hug the actual counts.
    order = sorted(range(E_RT), key=lambda e: -len(ids[e]))
    slot_experts = {0: order[:CORES], 1: order[CORES:]}
    caps = []
    for slot in range(2):
        mx = max(len(ids[e]) for e in slot_experts[slot])
        caps.append(max(512, -(-mx // 16) * 16))
    r0_cap, r1_cap = caps

    key = (r0_cap, r1_cap)
    if key not in _prog_cache:
        _prog_cache[key] = _build_program(r0_cap, r1_cap)
    nc = _prog_cache[key]

    xbf = x2d.astype(NP_BF16)
    x8 = x2d.astype(NP_FP8)  # |x| < 6 << 240: no clipping needed

    in_maps = []
    for c in range(CORES):
        se, dh, th = c % E_SH, (c // E_SH) % 2, c // 4
        m = {
            "x_sh": _pack_x(np.ascontiguousarray(
                xbf[th * N_SH:(th + 1) * N_SH].T)),
            "wup_sh": _pack_rows(np.ascontiguousarray(
                Wsh_up[se][:, dh * D_SH:(dh + 1) * D_SH]).astype(NP_BF16),
                HT),
            "wdn_sh": _pack_rows(np.ascontiguousarray(
                Wsh_down[se][dh * D_SH:(dh + 1) * D_SH, :]).astype(NP_BF16),
                DT_SH),
        }
        for slot, cap in ((0, r0_cap), (1, r1_cap)):
            e = slot_experts[slot][c]
            sel = ids[e]
            xe = np.zeros((H, cap), NP_FP8)
            xe[:, :len(sel)] = x8[sel].T
            m[f"x_r{slot}"] = _pack_x(xe)
            wu8 = (Wrt_up[e] * WS).astype(NP_FP8)
            # D-half-major layout: [2, HT, 512] per partition (see
            # load_wup_dh)
            m[f"wup_r{slot}"] = _pack_rows(
                np.ascontiguousarray(np.vstack([wu8[:, :512],
                                                wu8[:, 512:]])), 2 * HT)
            m[f"wdn_r{slot}"] = _pack_rows(
                (Wrt_down[e] * WS).astype(NP_FP8), DT_RT)
        in_maps.append(m)

    res = run_bass_kernel_spmd(nc, in_maps, core_ids=list(range(CORES)))
    LAST_RESULTS = res

    out = np.zeros((T, H), np.float32)
    for c in range(CORES):
        th = c // 4
        out[th * N_SH:(th + 1) * N_SH] += _unpack_y(res.results[c]["y_sh"],
                                                    N_SH)
    for slot, cap in ((0, r0_cap), (1, r1_cap)):
        for c in range(CORES):
            e = slot_experts[slot][c]
            sel = ids[e]
            y = _unpack_y(res.results[c][f"y_r{slot}"], cap)
            out[sel] += (gates[e][:, None] / WS) * y[:len(sel)]
    return out.reshape(B, S, H)


# revision 37
# speedup vs baseline: 1.0217x; 1.0012x over previous
"""MoE (top-2 of 16 routed experts + 2 shared experts) Trainium2 kernel.

Strategy: expert-parallel over 8 NeuronCores, token dispatch on host (the
router is 0.01% of the FLOPs; host-side routing lets each core receive
exactly the tokens it needs, already gathered, transposed and packed).

Per core (SPMD program, identical on all cores; per-core in_maps differ):
  slot "r1": routed expert (one of the 8 least-loaded), fp8 DoubleRow
  slot "r0": routed expert (one of the 8 most-loaded), fp8 DoubleRow
  slot "sh": one shared expert x one D-half x one token-half (2048 tokens),
             bf16
Each slot is a dense FFN in feature-major ("transposed") layout:
  mid^T[d,t] = gelu( sum_h Wup[h,d] * x^T[h,t] )
  y^T[h,t]   =       sum_d Wdn[d,h] * mid^T[d,t]
Host scatter-adds y^T into the [T,H] output with the router gate weights
(gelu is the only nonlinearity, so the per-token gate commutes with the
down projection).

Numerics: the routed contribution is ~19% of output energy, so fp8-e4m3
there (per-expert rel err ~5%) adds only ~1% to the total error; shared
experts stay bf16.  Routed weights are pre-scaled x32 into e4m3's normal
range; the up-projection's x32 is undone by the gelu activation scale,
the down-projection's x32 by dividing the host-side gate.

fp8 matmuls run in DoubleRow perf mode: lhsT [128,2,128] + rhs [128,2,w]
process a 256-deep contraction per instruction at 2 MAC/cell/cycle.  The
tile-major SBUF packing already pairs correctly: [:, 2b:2b+2, :] puts
contraction rows 256b+p and 256b+128+p on partition p.  Chunks of one
routed slot are interleaved in the b-loop so consecutive matmuls share a
stationary operand and LDWEIGHTS (~183ns for 256 cols) stays hidden.

All DRAM tensors are packed host-side as [128, free] with each partition's
data contiguous (x/y additionally chunk-major), so every DMA moves large
contiguous blocks per partition.
"""

import numpy as np
import ml_dtypes

import concourse.mybir as mybir
import concourse.tile as tile
from concourse import bacc
from concourse.bass_utils import run_bass_kernel_spmd

BF16 = mybir.dt.bfloat16
FP8 = mybir.dt.float8e4
F32 = mybir.dt.float32
NP_BF16 = ml_dtypes.bfloat16
NP_FP8 = ml_dtypes.float8_e4m3  # IEEE e4m3 (max 240) = TRN FP8_EXP4
GELU = mybir.ActivationFunctionType.Gelu
DR = mybir.MatmulPerfMode.DoubleRow

B, S, H, D = 4, 1024, 2048, 1024
T = B * S
E_RT, E_SH, CORES = 16, 2, 8
HT = H // 128        # contraction tiles for every up-projection (16)
DT_RT = D // 128     # routed intermediate tiles (8)
D_SH = D // 2        # per-core shared intermediate columns (512)
DT_SH = D_SH // 128  # 4
N_SH = T // 2        # shared-slot tokens per core (2048)
TT = 512             # max moving-operand / psum tile width
YG = 4               # output h-tiles staged per store DMA
WS = 32.0            # fp8 weight pre-scale (power of two)

_prog_cache = {}
LAST_RESULTS = None  # BassKernelResults of the most recent run (for test.py)


def _chunks(n):
    """Split n greedily into TT-wide chunks plus a runt.  Full-width
    chunks maximize matmul efficiency, and the single psum->y cast per
    512-wide chunk is cheaper on DVE than two medium casts."""
    return [(off, min(TT, n - off)) for off in range(0, n, TT)]


def _build_program(r0_cap, r1_cap):
    nc = bacc.Bacc("TRN2", target_bir_lowering=False, debug=False,
                   num_devices=CORES)
    # (name, n_tokens, fp8, dts);  contraction tiles are HT for all slots.
    # Order: r1 first (smallest load -> earliest compute start), sh in the
    # middle (its 12.6MB of loads hide under r1's compute), r0 last (its
    # final 288-wide chunks drain the tail fastest).
    slot_defs = [("r1", r1_cap, True, DT_RT),
                 ("sh", N_SH, False, DT_SH),
                 ("r0", r0_cap, True, DT_RT)]
    slots = []
    for name, n, fp8, dts in slot_defs:
        dt = FP8 if fp8 else BF16
        d_s = dts * 128
        xd = nc.dram_tensor(f"x_{name}", [128, HT * n], dt,
                            kind="ExternalInput")
        wu = nc.dram_tensor(f"wup_{name}", [128, HT * d_s], dt,
                            kind="ExternalInput")
        wd = nc.dram_tensor(f"wdn_{name}", [128, dts * H], dt,
                            kind="ExternalInput")
        yd = nc.dram_tensor(f"y_{name}", [128, HT * n], BF16,
                            kind="ExternalOutput")
        slots.append((name, n, fp8, dts, xd, wu, wd, yd))

    with tile.TileContext(nc) as tc:
        with (
            tc.tile_pool(name="wpool", bufs=1) as wpool,
            tc.tile_pool(name="xpool", bufs=1) as xpool,
            tc.tile_pool(name="mpool", bufs=1) as mpool,
            tc.tile_pool(name="ypool", bufs=4) as ypool,
            tc.tile_pool(name="pspool", bufs=8, space="PSUM") as pspool,
        ):
            # DMA orchestration.  Two HWDGE rings exist (issued via SP and
            # ACT); each processes its DMAs in issue order, and an issue
            # blocks while its destination pool slot is busy.  The rings
            # run at ~180 GB/s each (HBM aggregate ~360), so the startup
            # critical path is balanced across both: r1's x+wup split
            # evenly, then sh's critical pieces, then r0's loads with
            # sh's trailing x chunks interleaved behind them.  The ACT
            # ring additionally carries all y stores.  Weight/x buffers
            # are statically dedicated per slot (per-slot tags) except
            # sh's x, which round-robins 3 buffers.
            def slot_io(si):
                name, n, fp8, dts, xd, wu, wd, yd = slots[si]
                dt = FP8 if fp8 else BF16
                d_s = dts * 128
                chs = _chunks(n)
                wut = wpool.tile([128, HT, d_s], dt, tag=f"wup_{name}",
                                 name=f"wup_{name}")
                wdt = wpool.tile([128, dts, H], dt, tag=f"wdn_{name}",
                                 name=f"wdn_{name}")
                xts = [None] * len(chs)

                def load_x(ci, eng, h0=0, hn=HT):
                    off, w = chs[ci]
                    if xts[ci] is None:
                        bufs = 3 if name == "sh" else 1
                        xtag = (f"x_{name}" if name == "sh"
                                else f"x_{name}_{ci}")
                        xts[ci] = xpool.tile([128, HT, w], dt, tag=xtag,
                                             bufs=bufs,
                                             name=f"x_{name}_{off}")
                    eng.dma_start(
                        out=xts[ci][:, h0:h0 + hn, :],
                        in_=xd[:, HT * off + h0 * w:
                               HT * off + (h0 + hn) * w].rearrange(
                            "p (h w) -> p h w", h=hn))

                def load_wup(gi, eng, hg=4):
                    g = gi * hg
                    eng.dma_start(
                        out=wut[:, g:g + hg, :],
                        in_=wu[:, g * d_s:(g + hg) * d_s].rearrange(
                            "p (h d) -> p h d", h=hg))

                def load_wup_dh(dh, gi, eng, hg=4):
                    # fp8 wup DRAM layout is D-half-major ([2, HT, 512] per
                    # partition) so the first up-chains only need half the
                    # weight bytes before they can run to completion.
                    g = gi * hg
                    base = (dh * HT + g) * 512
                    eng.dma_start(
                        out=wut[:, g:g + hg, dh * 512:(dh + 1) * 512],
                        in_=wu[:, base:base + hg * 512].rearrange(
                            "p (h d) -> p h d", h=hg))

                def load_wdn(gi, eng):
                    dg = 4
                    g = gi * dg
                    eng.dma_start(
                        out=wdt[:, g:g + dg, :],
                        in_=wd[:, g * H:(g + dg) * H].rearrange(
                            "p (c h) -> p c h", c=dg))

                return (wut, wdt, xts, load_x, load_wup, load_wdn,
                        load_wup_dh)

            # Dummy matmuls on scratch tiles: the PE HAM clock-gate only
            # lifts to 2.4 GHz after ~3.4us of sustained activity, so warm
            # it up while the first loads stream in.
            wlhs = xpool.tile([128, 128], BF16, tag="warm_l", bufs=1,
                              name="warm_lhs")
            wrhs = xpool.tile([128, TT], BF16, tag="warm_r", bufs=1,
                              name="warm_rhs")
            nc.vector.memset(wlhs[:], 0)
            nc.vector.memset(wrhs[:], 0)
            wps = pspool.tile([128, TT], F32, tag="ps", name="warm_ps")
            for wi in range(20):
                nc.tensor.matmul(wps[:], lhsT=wlhs[:], rhs=wrhs[:],
                                 start=True, stop=True)

            sio = [slot_io(si) for si in range(len(slots))]
            (_, _, _, r1_x, r1_wup, r1_wdn, _) = sio[0]
            (_, _, _, sh_x, sh_wup, sh_wdn, _) = sio[1]
            (_, _, _, r0_x, r0_wup, r0_wdn, _) = sio[2]
            hh = HT // 2
            # r1 (first slot): x quarters + full-D wup groups alternated
            # across both rings in b-outer consumption order, so its
            # streamed up phase runs as the loads arrive; wdn last
            r1_x(0, nc.sync, 0, 4)
            r1_wup(0, nc.scalar)
            r1_wup(1, nc.sync)
            r1_x(0, nc.scalar, 4, 4)
            r1_x(0, nc.sync, 8, 4)
            r1_wup(2, nc.scalar)
            r1_wup(3, nc.sync)
            r1_x(0, nc.scalar, 12, 4)
            r1_wdn(0, nc.sync)
            r1_wdn(1, nc.scalar)
            # sh: critical pieces (wup + x chunk 0) split across rings
            sh_wup(0, nc.sync)
            sh_wup(1, nc.scalar)
            sh_wup(2, nc.sync)
            sh_wup(3, nc.scalar)
            sh_x(0, nc.sync, hh, HT - hh)
            sh_x(0, nc.scalar, 0, hh)
            sh_wdn(0, nc.sync)
            sh_x(1, nc.sync)
            # r0's loads, with sh's trailing x chunks interleaved behind
            for gi in range(4):
                r0_wup(gi, nc.sync)
            r0_x(0, nc.sync)
            sh_x(2, nc.sync)
            r0_wdn(0, nc.sync)
            r0_wdn(1, nc.sync)
            r0_x(1, nc.sync)
            sh_x(3, nc.sync)

            def store_y(yd, yt, off, hi, w):
                lo = HT * off + (hi - (YG - 1)) * w
                nc.scalar.dma_start(
                    out=yd[:, lo:lo + YG * w].rearrange(
                        "p (h w) -> p h w", h=YG),
                    in_=yt[:])

            def routed_compute(name, n, dts, xts, wut, wdt, yd,
                               last=False, first=False):
                chs = _chunks(n)
                nch = len(chs)
                mids = [mpool.tile([128, dts, w], FP8, tag=f"mid_{name}",
                                   bufs=nch, name=f"mid_{name}_{off}")
                        for off, w in chs]
                hb = HT // 2
                if first and nch == 1:
                    # First slot: its up phase is DMA-paced, so stream it
                    # b-outer with all dts psum chains live (exactly the
                    # 8 PSUM banks) -- each matmul then needs only one x
                    # quarter + one wup group, overlapping the whole load.
                    # The last hb/2 b-steps run per-dj so chains complete
                    # staggered (864ns apart >= one 690ns gelu) and the
                    # down phase never waits on a gelu backlog.
                    off, w = chs[0]
                    xt = xts[0]
                    pss = [pspool.tile([128, TT], F32, tag="ps",
                                        name=f"p1_{name}_{dj}")
                           for dj in range(dts)]
                    hb0 = hb // 2
                    for b in range(hb0):
                        for dj in range(dts):
                            nc.tensor.matmul(
                                pss[dj][:, :w],
                                lhsT=wut[:, 2 * b:2 * b + 2,
                                         dj * 128:(dj + 1) * 128],
                                rhs=xt[:, 2 * b:2 * b + 2, :],
                                start=(b == 0), stop=False,
                                perf_mode=DR)
                    for dj in range(dts):
                        for b in range(hb0, hb):
                            nc.tensor.matmul(
                                pss[dj][:, :w],
                                lhsT=wut[:, 2 * b:2 * b + 2,
                                         dj * 128:(dj + 1) * 128],
                                rhs=xt[:, 2 * b:2 * b + 2, :],
                                start=False, stop=(b == hb - 1),
                                perf_mode=DR)
                        nc.scalar.activation(mids[0][:, dj, :],
                                             pss[dj][:, :w], GELU,
                                             scale=1.0 / WS)
                else:
                    for dj in range(dts):
                        pss = [pspool.tile([128, TT], F32, tag="ps",
                                            name=f"p1_{name}_{dj}_{ci}")
                               for ci in range(nch)]
                        for b in range(hb):
                            for ci, (off, w) in enumerate(chs):
                                nc.tensor.matmul(
                                    pss[ci][:, :w],
                                    lhsT=wut[:, 2 * b:2 * b + 2,
                                             dj * 128:(dj + 1) * 128],
                                    rhs=xts[ci][:, 2 * b:2 * b + 2, :],
                                    start=(b == 0), stop=(b == hb - 1),
                                    perf_mode=DR)
                        for ci, (off, w) in enumerate(chs):
                            nc.scalar.activation(mids[ci][:, dj, :],
                                                 pss[ci][:, :w], GELU,
                                                 scale=1.0 / WS)
                db = dts // 2
                yts = [None] * nch
                for hi in range(HT):
                    ps2s = [pspool.tile([128, TT], F32, tag="ps",
                                         name=f"p2_{name}_{hi}_{ci}")
                            for ci in range(nch)]
                    for c in range(db):
                        for ci, (off, w) in enumerate(chs):
                            nc.tensor.matmul(
                                ps2s[ci][:, :w],
                                lhsT=wdt[:, 2 * c:2 * c + 2,
                                         hi * 128:(hi + 1) * 128],
                                rhs=mids[ci][:, 2 * c:2 * c + 2, :],
                                start=(c == 0), stop=(c == db - 1),
                                perf_mode=DR)
                    g = hi % YG
                    for ci, (off, w) in enumerate(chs):
                        if g == 0:
                            yts[ci] = ypool.tile([128, YG, w], BF16,
                                                 tag="y",
                                                 name=f"y_{name}_{hi}_{ci}")
                        nc.vector.tensor_copy(yts[ci][:, g, :],
                                              ps2s[ci][:, :w])
                        if last and hi >= HT - YG:
                            # final group of the final slot: store in
                            # half-groups so the kernel's last store DMA
                            # is smaller and the end-of-kernel
                            # DMA-completion barrier fires sooner
                            if g % 2:
                                lo = HT * off + (hi - 1) * w
                                nc.scalar.dma_start(
                                    out=yd[:, lo:lo + 2 * w].rearrange(
                                        "p (h w) -> p h w", h=2),
                                    in_=yts[ci][:, g - 1:g + 1, :])
                        elif g == YG - 1:
                            store_y(yd, yts[ci], off, hi, w)

            def shared_compute(name, n, dts, xts, wut, wdt, yd):
                for ci, (off, w) in enumerate(_chunks(n)):
                    xt = xts[ci]
                    mid = mpool.tile([128, dts, w], BF16, tag="mid_sh",
                                     bufs=2, name=f"mid_{name}_{off}")
                    for dj in range(dts):
                        ps = pspool.tile([128, TT], F32, tag="ps",
                                          name=f"p1_{name}_{off}_{dj}")
                        for hi in range(HT):
                            nc.tensor.matmul(
                                ps[:, :w],
                                lhsT=wut[:, hi, dj * 128:(dj + 1) * 128],
                                rhs=xt[:, hi, :],
                                start=(hi == 0), stop=(hi == HT - 1))
                        nc.scalar.activation(mid[:, dj, :], ps[:, :w],
                                             GELU)
                    yt = None
                    for hi in range(HT):
                        ps2 = pspool.tile([128, TT], F32, tag="ps",
                                           name=f"p2_{name}_{off}_{hi}")
                        for dj in range(dts):
                            nc.tensor.matmul(
                                ps2[:, :w],
                                lhsT=wdt[:, dj, hi * 128:(hi + 1) * 128],
                                rhs=mid[:, dj, :],
                                start=(dj == 0), stop=(dj == dts - 1))
                        g = hi % YG
                        if g == 0:
                            yt = ypool.tile([128, YG, w], BF16, tag="y",
                                            name=f"y_{name}_{off}_{hi}")
                        if hi % 2:
                            nc.scalar.copy(yt[:, g, :], ps2[:, :w])
                        else:
                            nc.vector.tensor_copy(yt[:, g, :], ps2[:, :w])
                        if g == YG - 1:
                            store_y(yd, yt, off, hi, w)

            for si, (name, n, fp8, dts, xd, wu, wd, yd) in enumerate(slots):
                wut, wdt, xts = sio[si][:3]
                if fp8:
                    routed_compute(name, n, dts, xts, wut, wdt, yd,
                                   last=(si == len(slots) - 1),
                                   first=(si == 0))
                else:
                    shared_compute(name, n, dts, xts, wut, wdt, yd)
    nc.compile()
    return nc


def _pack_rows(a, nt):
    """[nt*128, m] row-major -> [128, nt*m] with per-partition contiguous
    (tile-major) layout."""
    m = a.shape[1]
    return np.ascontiguousarray(
        a.reshape(nt, 128, m).transpose(1, 0, 2).reshape(128, nt * m))


def _pack_x(xTc):
    """[H, n] -> [128, HT*n] chunk-major."""
    n = xTc.shape[1]
    parts = [_pack_rows(xTc[:, off:off + w], HT) for off, w in _chunks(n)]
    return np.ascontiguousarray(np.concatenate(parts, axis=1))


def _unpack_y(yflat, n):
    """[128, HT*n] chunk-major -> [n, H] (token-major)."""
    yflat = yflat.astype(np.float32)
    out = np.empty((n, H), np.float32)
    base = 0
    for off, w in _chunks(n):
        blk = yflat[:, base:base + HT * w].reshape(128, HT, w)
        out[off:off + w] = blk.transpose(2, 1, 0).reshape(w, H)
        base += HT * w
    return out


def _route(x2d, w_router):
    """Top-2 routing, matching the reference's softmax-then-top_k."""
    logits = x2d @ w_router
    m = logits.max(-1, keepdims=True)
    e = np.exp(logits - m)
    probs = e / e.sum(-1, keepdims=True)
    rows = np.arange(x2d.shape[0])
    i1 = probs.argmax(-1)
    masked = probs.copy()
    masked[rows, i1] = -np.inf
    i2 = masked.argmax(-1)
    return probs, i1, i2


def kernel(x, Wsh_up, Wsh_down, Wrt_up, Wrt_down, W_router):
    global LAST_RESULTS
    x = np.asarray(x, np.float32)
    Wsh_up = np.asarray(Wsh_up, np.float32)
    Wsh_down = np.asarray(Wsh_down, np.float32)
    Wrt_up = np.asarray(Wrt_up, np.float32)
    Wrt_down = np.asarray(Wrt_down, np.float32)
    W_router = np.asarray(W_router, np.float32)

    x2d = x.reshape(T, H)
    probs, i1, i2 = _route(x2d, W_router)

    # token ids / gate values per routed expert
    ids, gates = [], []
    for e in range(E_RT):
        sel = np.where((i1 == e) | (i2 == e))[0]
        ids.append(sel)
        gates.append(probs[sel, e].astype(np.float32))

    # slot r0 takes the 8 most-loaded experts, r1 the 8 least-loaded, so
    # the two static capacities hug the actual counts.
    order = sorted(range(E_RT), key=lambda e: -len(ids[e]))
    slot_experts = {0: order[:CORES], 1: order[CORES:]}
    caps = []
    for slot in range(2):
        mx = max(len(ids[e]) for e in slot_experts[slot])
        caps.append(max(512, -(-mx // 16) * 16))
    r0_cap, r1_cap = caps

    key = (r0_cap, r1_cap)
    if key not in _prog_cache:
        _prog_cache[key] = _build_program(r0_cap, r1_cap)
    nc = _prog_cache[key]

    xbf = x2d.astype(NP_BF16)
    x8 = x2d.astype(NP_FP8)  # |x| < 6 << 240: no clipping needed

    in_maps = []
    for c in range(CORES):
        se, dh, th = c % E_SH, (c // E_SH) % 2, c // 4
        m = {
            "x_sh": _pack_x(np.ascontiguousarray(
                xbf[th * N_SH:(th + 1) * N_SH].T)),
            "wup_sh": _pack_rows(np.ascontiguousarray(
                Wsh_up[se][:, dh * D_SH:(dh + 1) * D_SH]).astype(NP_BF16),
                HT),
            "wdn_sh": _pack_rows(np.ascontiguousarray(
                Wsh_down[se][dh * D_SH:(dh + 1) * D_SH, :]).astype(NP_BF16),
                DT_SH),
        }
        for slot, cap in ((0, r0_cap), (1, r1_cap)):
            e = slot_experts[slot][c]
            sel = ids[e]
            xe = np.zeros((H, cap), NP_FP8)
            xe[:, :len(sel)] = x8[sel].T
            m[f"x_r{slot}"] = _pack_x(xe)
            m[f"wup_r{slot}"] = _pack_rows(
                (Wrt_up[e] * WS).astype(NP_FP8), HT)
            m[f"wdn_r{slot}"] = _pack_rows(
                (Wrt_down[e] * WS).astype(NP_FP8), DT_RT)
        in_maps.append(m)

    res = run_bass_kernel_spmd(nc, in_maps, core_ids=list(range(CORES)))
    LAST_RESULTS = res

    out = np.zeros((T, H), np.float32)
    for c in range(CORES):
        th = c // 4
        out[th * N_SH:(th + 1) * N_SH] += _unpack_y(res.results[c]["y_sh"],
                                                    N_SH)
    for slot, cap in ((0, r0_cap), (1, r1_cap)):
        for c in range(CORES):
            e = slot_experts[slot][c]
            sel = ids[e]
            y = _unpack_y(res.results[c][f"y_r{slot}"], cap)
            out[sel] += (gates[e][:, None] / WS) * y[:len(sel)]
    return out.reshape(B, S, H)


# revision 39
# speedup vs baseline: 1.0423x; 1.0201x over previous
"""MoE (top-2 of 16 routed experts + 2 shared experts) Trainium2 kernel.

Strategy: expert-parallel over 8 NeuronCores, token dispatch on host (the
router is 0.01% of the FLOPs; host-side routing lets each core receive
exactly the tokens it needs, already gathered, transposed and packed).

Per core (SPMD program, identical on all cores; per-core in_maps differ):
  slot "r1": routed expert (one of the 8 least-loaded), fp8 DoubleRow
  slot "r0": routed expert (one of the 8 most-loaded), fp8 DoubleRow
  slot "sh": one shared expert x one D-half x one token-half (2048 tokens),
             bf16
Each slot is a dense FFN in feature-major ("transposed") layout:
  mid^T[d,t] = gelu( sum_h Wup[h,d] * x^T[h,t] )
  y^T[h,t]   =       sum_d Wdn[d,h] * mid^T[d,t]
Host scatter-adds y^T into the [T,H] output with the router gate weights
(gelu is the only nonlinearity, so the per-token gate commutes with the
down projection).

Numerics: the routed contribution is ~19% of output energy, so fp8-e4m3
there (per-expert rel err ~5%) adds only ~1% to the total error; shared
experts stay bf16.  Routed weights are pre-scaled x32 into e4m3's normal
range; the up-projection's x32 is undone by the gelu activation scale,
the down-projection's x32 by dividing the host-side gate.

fp8 matmuls run in DoubleRow perf mode: lhsT [128,2,128] + rhs [128,2,w]
process a 256-deep contraction per instruction at 2 MAC/cell/cycle.  The
tile-major SBUF packing already pairs correctly: [:, 2b:2b+2, :] puts
contraction rows 256b+p and 256b+128+p on partition p.  Chunks of one
routed slot are interleaved in the b-loop so consecutive matmuls share a
stationary operand and LDWEIGHTS (~183ns for 256 cols) stays hidden.

All DRAM tensors are packed host-side as [128, free] with each partition's
data contiguous (x/y additionally chunk-major), so every DMA moves large
contiguous blocks per partition.
"""

import numpy as np
import ml_dtypes

import concourse.mybir as mybir
import concourse.tile as tile
from concourse import bacc
from concourse.bass_utils import run_bass_kernel_spmd

BF16 = mybir.dt.bfloat16
FP8 = mybir.dt.float8e4
F32 = mybir.dt.float32
NP_BF16 = ml_dtypes.bfloat16
NP_FP8 = ml_dtypes.float8_e4m3  # IEEE e4m3 (max 240) = TRN FP8_EXP4
GELU = mybir.ActivationFunctionType.Gelu
DR = mybir.MatmulPerfMode.DoubleRow

B, S, H, D = 4, 1024, 2048, 1024
T = B * S
E_RT, E_SH, CORES = 16, 2, 8
HT = H // 128        # contraction tiles for every up-projection (16)
DT_RT = D // 128     # routed intermediate tiles (8)
D_SH = D // 2        # per-core shared intermediate columns (512)
DT_SH = D_SH // 128  # 4
N_SH = T // 2        # shared-slot tokens per core (2048)
TT = 512             # max moving-operand / psum tile width
YG = 4               # output h-tiles staged per store DMA
WS = 32.0            # fp8 weight pre-scale (power of two)

_prog_cache = {}
LAST_RESULTS = None  # BassKernelResults of the most recent run (for test.py)


def _chunks(n):
    """Split n greedily into TT-wide chunks plus a runt.  Full-width
    chunks maximize matmul efficiency, and the single psum->y cast per
    512-wide chunk is cheaper on DVE than two medium casts."""
    return [(off, min(TT, n - off)) for off in range(0, n, TT)]


def _build_program(r0_cap, r1_cap):
    nc = bacc.Bacc("TRN2", target_bir_lowering=False, debug=False,
                   num_devices=CORES)
    # (name, n_tokens, fp8, dts);  contraction tiles are HT for all slots.
    # Order: r1 first (smallest load -> earliest compute start), sh in the
    # middle (its 12.6MB of loads hide under r1's compute), r0 last (its
    # final 288-wide chunks drain the tail fastest).
    slot_defs = [("r1", r1_cap, True, DT_RT),
                 ("sh", N_SH, False, DT_SH),
                 ("r0", r0_cap, True, DT_RT)]
    slots = []
    for name, n, fp8, dts in slot_defs:
        dt = FP8 if fp8 else BF16
        d_s = dts * 128
        xd = nc.dram_tensor(f"x_{name}", [128, HT * n], dt,
                            kind="ExternalInput")
        wu = nc.dram_tensor(f"wup_{name}", [128, HT * d_s], dt,
                            kind="ExternalInput")
        wd = nc.dram_tensor(f"wdn_{name}", [128, dts * H], dt,
                            kind="ExternalInput")
        yd = nc.dram_tensor(f"y_{name}", [128, HT * n], BF16,
                            kind="ExternalOutput")
        slots.append((name, n, fp8, dts, xd, wu, wd, yd))

    with tile.TileContext(nc) as tc:
        with (
            tc.tile_pool(name="wpool", bufs=1) as wpool,
            tc.tile_pool(name="xpool", bufs=1) as xpool,
            tc.tile_pool(name="mpool", bufs=1) as mpool,
            tc.tile_pool(name="ypool", bufs=4) as ypool,
            tc.tile_pool(name="pspool", bufs=8, space="PSUM") as pspool,
        ):
            # DMA orchestration.  Two HWDGE rings exist (issued via SP and
            # ACT); each processes its DMAs in issue order, and an issue
            # blocks while its destination pool slot is busy.  The rings
            # run at ~180 GB/s each (HBM aggregate ~360), so the startup
            # critical path is balanced across both: r1's x+wup split
            # evenly, then sh's critical pieces, then r0's loads with
            # sh's trailing x chunks interleaved behind them.  The ACT
            # ring additionally carries all y stores.  Weight/x buffers
            # are statically dedicated per slot (per-slot tags) except
            # sh's x, which round-robins 3 buffers.
            def slot_io(si):
                name, n, fp8, dts, xd, wu, wd, yd = slots[si]
                dt = FP8 if fp8 else BF16
                d_s = dts * 128
                chs = _chunks(n)
                wut = wpool.tile([128, HT, d_s], dt, tag=f"wup_{name}",
                                 name=f"wup_{name}")
                wdt = wpool.tile([128, dts, H], dt, tag=f"wdn_{name}",
                                 name=f"wdn_{name}")
                xts = [None] * len(chs)

                def load_x(ci, eng, h0=0, hn=HT):
                    off, w = chs[ci]
                    if xts[ci] is None:
                        bufs = 3 if name == "sh" else 1
                        xtag = (f"x_{name}" if name == "sh"
                                else f"x_{name}_{ci}")
                        xts[ci] = xpool.tile([128, HT, w], dt, tag=xtag,
                                             bufs=bufs,
                                             name=f"x_{name}_{off}")
                    eng.dma_start(
                        out=xts[ci][:, h0:h0 + hn, :],
                        in_=xd[:, HT * off + h0 * w:
                               HT * off + (h0 + hn) * w].rearrange(
                            "p (h w) -> p h w", h=hn))

                def load_wup(gi, eng, hg=4):
                    g = gi * hg
                    eng.dma_start(
                        out=wut[:, g:g + hg, :],
                        in_=wu[:, g * d_s:(g + hg) * d_s].rearrange(
                            "p (h d) -> p h d", h=hg))

                def load_wup_dh(dh, gi, eng, hg=4):
                    # fp8 wup DRAM layout is D-half-major ([2, HT, 512] per
                    # partition) so the first up-chains only need half the
                    # weight bytes before they can run to completion.
                    g = gi * hg
                    base = (dh * HT + g) * 512
                    eng.dma_start(
                        out=wut[:, g:g + hg, dh * 512:(dh + 1) * 512],
                        in_=wu[:, base:base + hg * 512].rearrange(
                            "p (h d) -> p h d", h=hg))

                def load_wdn(gi, eng):
                    dg = 4
                    g = gi * dg
                    eng.dma_start(
                        out=wdt[:, g:g + dg, :],
                        in_=wd[:, g * H:(g + dg) * H].rearrange(
                            "p (c h) -> p c h", c=dg))

                def load_wdn_hh(half, eng):
                    # all d-tiles, one H-column half: the down phase
                    # consumes wdn columns in hi order, so the first half
                    # unblocks down chains hi 0..HT/2-1
                    hw_ = H // 2
                    eng.dma_start(
                        out=wdt[:, :, half * hw_:(half + 1) * hw_],
                        in_=wd[:, :].rearrange(
                            "p (c h) -> p c h",
                            c=dts)[:, :, half * hw_:(half + 1) * hw_])

                return (wut, wdt, xts, load_x, load_wup, load_wdn,
                        load_wdn_hh)

            # Dummy matmuls on scratch tiles: the PE HAM clock-gate only
            # lifts to 2.4 GHz after ~3.4us of sustained activity, so warm
            # it up while the first loads stream in.
            wlhs = xpool.tile([128, 128], BF16, tag="warm_l", bufs=1,
                              name="warm_lhs")
            wrhs = xpool.tile([128, TT], BF16, tag="warm_r", bufs=1,
                              name="warm_rhs")
            nc.vector.memset(wlhs[:], 0)
            nc.vector.memset(wrhs[:], 0)
            wps = pspool.tile([128, TT], F32, tag="ps", name="warm_ps")
            for wi in range(20):
                nc.tensor.matmul(wps[:], lhsT=wlhs[:], rhs=wrhs[:],
                                 start=True, stop=True)

            sio = [slot_io(si) for si in range(len(slots))]
            (_, _, _, r1_x, r1_wup, _, r1_wdnh) = sio[0]
            (_, _, _, sh_x, sh_wup, sh_wdn, _) = sio[1]
            (_, _, _, r0_x, r0_wup, r0_wdn, _) = sio[2]
            hh = HT // 2
            # r1 (first slot): x quarters + full-D wup groups alternated
            # across both rings in b-outer consumption order, so its
            # streamed up phase runs as the loads arrive; wdn last
            r1_x(0, nc.sync, 0, 4)
            r1_wup(0, nc.scalar)
            r1_wup(1, nc.sync)
            r1_x(0, nc.scalar, 4, 4)
            r1_x(0, nc.sync, 8, 4)
            r1_wup(2, nc.scalar)
            r1_wup(3, nc.sync)
            r1_x(0, nc.scalar, 12, 4)
            r1_wdnh(0, nc.sync)
            r1_wdnh(1, nc.scalar)
            # sh: critical pieces (wup + x chunk 0) split across rings
            sh_wup(0, nc.sync)
            sh_wup(1, nc.scalar)
            sh_wup(2, nc.sync)
            sh_wup(3, nc.scalar)
            sh_x(0, nc.sync, hh, HT - hh)
            sh_x(0, nc.scalar, 0, hh)
            sh_wdn(0, nc.sync)
            sh_x(1, nc.sync)
            # r0's loads, with sh's trailing x chunks interleaved behind
            for gi in range(4):
                r0_wup(gi, nc.sync)
            r0_x(0, nc.sync)
            sh_x(2, nc.sync)
            r0_wdn(0, nc.sync)
            r0_wdn(1, nc.sync)
            r0_x(1, nc.sync)
            sh_x(3, nc.sync)

            def store_y(yd, yt, off, hi, w):
                lo = HT * off + (hi - (YG - 1)) * w
                nc.scalar.dma_start(
                    out=yd[:, lo:lo + YG * w].rearrange(
                        "p (h w) -> p h w", h=YG),
                    in_=yt[:])

            def routed_compute(name, n, dts, xts, wut, wdt, yd,
                               last=False, first=False):
                chs = _chunks(n)
                nch = len(chs)
                mids = [mpool.tile([128, dts, w], FP8, tag=f"mid_{name}",
                                   bufs=nch, name=f"mid_{name}_{off}")
                        for off, w in chs]
                hb = HT // 2
                if first and nch == 1:
                    # First slot: its up phase is DMA-paced, so stream it
                    # b-outer with all dts psum chains live (exactly the
                    # 8 PSUM banks) -- each matmul then needs only one x
                    # quarter + one wup group, overlapping the whole load.
                    # The last hb/2 b-steps run per-dj so chains complete
                    # staggered (864ns apart >= one 690ns gelu) and the
                    # down phase never waits on a gelu backlog.
                    off, w = chs[0]
                    xt = xts[0]
                    pss = [pspool.tile([128, TT], F32, tag="ps",
                                        name=f"p1_{name}_{dj}")
                           for dj in range(dts)]
                    hb0 = hb // 2
                    for b in range(hb0):
                        for dj in range(dts):
                            nc.tensor.matmul(
                                pss[dj][:, :w],
                                lhsT=wut[:, 2 * b:2 * b + 2,
                                         dj * 128:(dj + 1) * 128],
                                rhs=xt[:, 2 * b:2 * b + 2, :],
                                start=(b == 0), stop=False,
                                perf_mode=DR)
                    for dj in range(dts):
                        for b in range(hb0, hb):
                            nc.tensor.matmul(
                                pss[dj][:, :w],
                                lhsT=wut[:, 2 * b:2 * b + 2,
                                         dj * 128:(dj + 1) * 128],
                                rhs=xt[:, 2 * b:2 * b + 2, :],
                                start=False, stop=(b == hb - 1),
                                perf_mode=DR)
                        nc.scalar.activation(mids[0][:, dj, :],
                                             pss[dj][:, :w], GELU,
                                             scale=1.0 / WS)
                else:
                    for dj in range(dts):
                        pss = [pspool.tile([128, TT], F32, tag="ps",
                                            name=f"p1_{name}_{dj}_{ci}")
                               for ci in range(nch)]
                        for b in range(hb):
                            for ci, (off, w) in enumerate(chs):
                                nc.tensor.matmul(
                                    pss[ci][:, :w],
                                    lhsT=wut[:, 2 * b:2 * b + 2,
                                             dj * 128:(dj + 1) * 128],
                                    rhs=xts[ci][:, 2 * b:2 * b + 2, :],
                                    start=(b == 0), stop=(b == hb - 1),
                                    perf_mode=DR)
                        for ci, (off, w) in enumerate(chs):
                            nc.scalar.activation(mids[ci][:, dj, :],
                                                 pss[ci][:, :w], GELU,
                                                 scale=1.0 / WS)
                db = dts // 2
                yts = [None] * nch
                for hi in range(HT):
                    ps2s = [pspool.tile([128, TT], F32, tag="ps",
                                         name=f"p2_{name}_{hi}_{ci}")
                            for ci in range(nch)]
                    for c in range(db):
                        for ci, (off, w) in enumerate(chs):
                            nc.tensor.matmul(
                                ps2s[ci][:, :w],
                                lhsT=wdt[:, 2 * c:2 * c + 2,
                                         hi * 128:(hi + 1) * 128],
                                rhs=mids[ci][:, 2 * c:2 * c + 2, :],
                                start=(c == 0), stop=(c == db - 1),
                                perf_mode=DR)
                    g = hi % YG
                    for ci, (off, w) in enumerate(chs):
                        if g == 0:
                            yts[ci] = ypool.tile([128, YG, w], BF16,
                                                 tag="y",
                                                 name=f"y_{name}_{hi}_{ci}")
                        nc.vector.tensor_copy(yts[ci][:, g, :],
                                              ps2s[ci][:, :w])
                        if last and hi >= HT - YG:
                            # final group of the final slot: store in
                            # half-groups so the kernel's last store DMA
                            # is smaller and the end-of-kernel
                            # DMA-completion barrier fires sooner
                            if g % 2:
                                lo = HT * off + (hi - 1) * w
                                nc.scalar.dma_start(
                                    out=yd[:, lo:lo + 2 * w].rearrange(
                                        "p (h w) -> p h w", h=2),
                                    in_=yts[ci][:, g - 1:g + 1, :])
                        elif g == YG - 1:
                            store_y(yd, yts[ci], off, hi, w)

            def shared_compute(name, n, dts, xts, wut, wdt, yd):
                for ci, (off, w) in enumerate(_chunks(n)):
                    xt = xts[ci]
                    mid = mpool.tile([128, dts, w], BF16, tag="mid_sh",
                                     bufs=2, name=f"mid_{name}_{off}")
                    for dj in range(dts):
                        ps = pspool.tile([128, TT], F32, tag="ps",
                                          name=f"p1_{name}_{off}_{dj}")
                        for hi in range(HT):
                            nc.tensor.matmul(
                                ps[:, :w],
                                lhsT=wut[:, hi, dj * 128:(dj + 1) * 128],
                                rhs=xt[:, hi, :],
                                start=(hi == 0), stop=(hi == HT - 1))
                        nc.scalar.activation(mid[:, dj, :], ps[:, :w],
                                             GELU)
                    yt = None
                    for hi in range(HT):
                        ps2 = pspool.tile([128, TT], F32, tag="ps",
                                           name=f"p2_{name}_{off}_{hi}")
                        for dj in range(dts):
                            nc.tensor.matmul(
                                ps2[:, :w],
                                lhsT=wdt[:, dj, hi * 128:(hi + 1) * 128],
                                rhs=mid[:, dj, :],
                                start=(dj == 0), stop=(dj == dts - 1))
                        g = hi % YG
                        if g == 0:
                            yt = ypool.tile([128, YG, w], BF16, tag="y",
                                            name=f"y_{name}_{off}_{hi}")
                        if hi % 2:
                            nc.scalar.copy(yt[:, g, :], ps2[:, :w])
                        else:
                            nc.vector.tensor_copy(yt[:, g, :], ps2[:, :w])
                        if g == YG - 1:
                            store_y(yd, yt, off, hi, w)

            for si, (name, n, fp8, dts, xd, wu, wd, yd) in enumerate(slots):
                wut, wdt, xts = sio[si][:3]
                if fp8:
                    routed_compute(name, n, dts, xts, wut, wdt, yd,
                                   last=(si == len(slots) - 1),
                                   first=(si == 0))
                else:
                    shared_compute(name, n, dts, xts, wut, wdt, yd)
    nc.compile()
    return nc


def _pack_rows(a, nt):
    """[nt*128, m] row-major -> [128, nt*m] with per-partition contiguous
    (tile-major) layout."""
    m = a.shape[1]
    return np.ascontiguousarray(
        a.reshape(nt, 128, m).transpose(1, 0, 2).reshape(128, nt * m))


def _pack_x(xTc):
    """[H, n] -> [128, HT*n] chunk-major."""
    n = xTc.shape[1]
    parts = [_pack_rows(xTc[:, off:off + w], HT) for off, w in _chunks(n)]
    return np.ascontiguousarray(np.concatenate(parts, axis=1))


def _unpack_y(yflat, n):
    """[128, HT*n] chunk-major -> [n, H] (token-major)."""
    yflat = yflat.astype(np.float32)
    out = np.empty((n, H), np.float32)
    base = 0
    for off, w in _chunks(n):
        blk = yflat[:, base:base + HT * w].reshape(128, HT, w)
        out[off:off + w] = blk.transpose(2, 1, 0).reshape(w, H)
        base += HT * w
    return out


def _route(x2d, w_router):
    """Top-2 routing, matching the reference's softmax-then-top_k."""
    logits = x2d @ w_router
    m = logits.max(-1, keepdims=True)
    e = np.exp(logits - m)
    probs = e / e.sum(-1, keepdims=True)
    rows = np.arange(x2d.shape[0])
    i1 = probs.argmax(-1)
    masked = probs.copy()
    masked[rows, i1] = -np.inf
    i2 = masked.argmax(-1)
    return probs, i1, i2


def kernel(x, Wsh_up, Wsh_down, Wrt_up, Wrt_down, W_router):
    global LAST_RESULTS
    x = np.asarray(x, np.float32)
    Wsh_up = np.asarray(Wsh_up, np.float32)
    Wsh_down = np.asarray(Wsh_down, np.float32)
    Wrt_up = np.asarray(Wrt_up, np.float32)
    Wrt_down = np.asarray(Wrt_down, np.float32)
    W_router = np.asarray(W_router, np.float32)

    x2d = x.reshape(T, H)
    probs, i1, i2 = _route(x2d, W_router)

    # token ids / gate values per routed expert
    ids, gates = [], []
    for e in range(E_RT):
        sel = np.where((i1 == e) | (i2 == e))[0]
        ids.append(sel)
        gates.append(probs[sel, e].astype(np.float32))

    # slot r0 takes the 8 most-loaded experts, r1 the 8 least-loaded, so
    # the two static capacities hug the actual counts.
    order = sorted(range(E_RT), key=lambda e: -len(ids[e]))
    slot_experts = {0: order[:CORES], 1: order[CORES:]}
    caps = []
    for slot in range(2):
        mx = max(len(ids[e]) for e in slot_experts[slot])
        caps.append(max(512, -(-mx // 16) * 16))
    r0_cap, r1_cap = caps

    key = (r0_cap, r1_cap)
    if key not in _prog_cache:
        _prog_cache[key] = _build_program(r0_cap, r1_cap)
    nc = _prog_cache[key]

    xbf = x2d.astype(NP_BF16)
    x8 = x2d.astype(NP_FP8)  # |x| < 6 << 240: no clipping needed

    in_maps = []
    for c in range(CORES):
        se, dh, th = c % E_SH, (c // E_SH) % 2, c // 4
        m = {
            "x_sh": _pack_x(np.ascontiguousarray(
                xbf[th * N_SH:(th + 1) * N_SH].T)),
            "wup_sh": _pack_rows(np.ascontiguousarray(
                Wsh_up[se][:, dh * D_SH:(dh + 1) * D_SH]).astype(NP_BF16),
                HT),
            "wdn_sh": _pack_rows(np.ascontiguousarray(
                Wsh_down[se][dh * D_SH:(dh + 1) * D_SH, :]).astype(NP_BF16),
                DT_SH),
        }
        for slot, cap in ((0, r0_cap), (1, r1_cap)):
            e = slot_experts[slot][c]
            sel = ids[e]
            xe = np.zeros((H, cap), NP_FP8)
            xe[:, :len(sel)] = x8[sel].T
            m[f"x_r{slot}"] = _pack_x(xe)
            m[f"wup_r{slot}"] = _pack_rows(
                (Wrt_up[e] * WS).astype(NP_FP8), HT)
            m[f"wdn_r{slot}"] = _pack_rows(
                (Wrt_down[e] * WS).astype(NP_FP8), DT_RT)
        in_maps.append(m)

    res = run_bass_kernel_spmd(nc, in_maps, core_ids=list(range(CORES)))
    LAST_RESULTS = res

    out = np.zeros((T, H), np.float32)
    for c in range(CORES):
        th = c // 4
        out[th * N_SH:(th + 1) * N_SH] += _unpack_y(res.results[c]["y_sh"],
                                                    N_SH)
    for slot, cap in ((0, r0_cap), (1, r1_cap)):
        for c in range(CORES):
            e = slot_experts[slot][c]
            sel = ids[e]
            y = _unpack_y(res.results[c][f"y_r{slot}"], cap)
            out[sel] += (gates[e][:, None] / WS) * y[:len(sel)]
    return out.reshape(B, S, H)
